# revision 17
# baseline (speedup 1.0000x reference)
"""CWCFace head (nn_CWCFace_11201274708637) — Trainium2 Bass kernel.

Math (reference):
    kn  = kernel / ||kernel||_col
    cos = clip(emb @ kn, -1+eps, 1-eps)              # [B, C]
    out = S * cos                                     # non-label columns
    out[i, label_i] = S * (cos(clip(arccos(cos) - M*ms_i)) - (M + M*ms_i))

Device work is the pure GEMM + epilogue:
    out = clip(emb @ (S * kernel / ||col||), -S(1-eps), +S(1-eps))   bf16
The column scale (S/||col||) is folded into the kernel matrix on the
host, and the B=512 label-column values (margin-adjusted cosines from
the per-class norm statistics) are patched into the assembled output
on the host — they are 0.001% of the output and pure [B]-sized math.

Sharding: classes column-split over 8 cores (model-parallel ArcFace),
CS = 8841 = ceil(70722/8) classes per core (zero-padded to 70728).

Per core: 18 class slices (17x512 + 137).  Per slice 4 B-tiles x
4 K-tiles of bf16 matmul accumulate [128, W] f32 PSUM tiles (8 banks,
two slices in flight), one fused DVE clip -> bf16 eviction per tile,
and one combined [128, 4, W] bf16 store per slice.

Engine-queue assignment matters: every engine pays a ~7us init-barrier
preamble before its first user instruction, each DMA-trigger costs
~650ns of serial issue time on its queue, and GpSimd DMAs go through
the slow software-descriptor path (Q0) — only Scalar and Sync issue
into the fast hardware DGE queues.  The head-critical embT /
kernel-slice-0 loads are k-chunked and interleaved across Scalar and
Sync, and slice 0 runs k-major against the arriving chunks.  All
steady-state traffic (kernel-block loads + stores) serializes on the
single Sync queue — two queues pushing into the shared 16-DMA-engine
pool interleave burstily and the resulting store latency jitter
ripples back (o_sb WAR -> eviction -> PSUM WAR) into PE stalls,
measured as ~1.5us/slice.  A zero-input warm-up matmul chain (PE user
code starts ~2.8us, before the DMA gate) ramps the PE p-state
(0.65 -> 2.4 GHz) so real matmuls start at full clock.
"""

import sys

for _p in (
    "/root/.axon_site",
    "/root/.axon_site/_ro/trn_rl_repo",
    "/root/.axon_site/_ro/pypackages",
    "/opt/trn_rl_repo",
):
    if _p not in sys.path:
        sys.path.append(_p)

import math

import numpy as np

import concourse.bass as bass
import concourse.mybir as mybir
import concourse.tile as tile
from concourse import bacc
from concourse.bass_utils import run_bass_kernel_spmd

B = 512
EMB = 512
C = 70722
NCORES = 8
CS = 8841  # ceil(C / NCORES); 8 * 8841 = 70728 >= 70722
S = 64.0
MARG = 0.4
H = 0.333
EPS = 1e-3

F32 = mybir.dt.float32
BF16 = mybir.dt.bfloat16
AL = mybir.AluOpType

KT = EMB // 128  # 4 K-tiles
BT = B // 128    # 4 B-tiles
CLIP = S * (1.0 - EPS)
N_WARMUP = 21    # PE p-state warm-up matmuls (no DMA dependency)


def _slices():
    """Class-column slices per core, widths <= 512 (one PSUM bank)."""
    out = []
    c0 = 0
    while c0 < CS:
        w = min(512, CS - c0)
        out.append((c0, w))
        c0 += w
    return out


def _emit(nc, tc, embT_h, kern_h, out_h):
    out3 = out_h[:, :].rearrange("(b p c) o -> p b (c o)", b=BT, p=128, c=CS)
    kernR = kern_h[:, :].rearrange("(k p) c -> p k c", p=128)  # [128, KT, CS]
    embR = embT_h[:, :].rearrange("(k p) b -> p k b", p=128)   # [128, KT, B]
    slices = _slices()
    NS = len(slices)

    # Blocks: slice 0 alone (k-chunked head load), two single-slice blocks
    # (fine-grained completion sems while the pipeline fills), then 2-slice
    # 1MB blocks.
    blocks = [[0], [1], [2]]
    si = 3
    while si < NS:
        blocks.append(list(range(si, min(si + 2, NS))))
        si += len(blocks[-1])
    NBLK = len(blocks)

    with (
        tc.tile_pool(name="cst", bufs=1) as cst,
        tc.tile_pool(name="kp", bufs=3) as kp,
        tc.tile_pool(name="op", bufs=5) as op_,
        tc.tile_pool(name="ps", bufs=8, space="PSUM") as ps,
    ):
        embT_sb = cst.tile([128, KT, B], BF16, tag="embT")
        zeros = cst.tile([128, 128 + 512], BF16, tag="zeros")
        nc.vector.memset(zeros[:], 0.0)

        ksbs = {}  # slice index -> (tile, offset)

        def load_block(bi):
            group = blocks[bi]
            c0 = slices[group[0]][0]
            Wb = sum(slices[s][1] for s in group)
            t = kp.tile([128, KT, 1024], BF16, tag="ks", name=f"ks{bi}")
            nc.sync.dma_start(out=t[:, :, :Wb], in_=kernR[:, :, c0 : c0 + Wb])
            off = 0
            for s in group:
                ksbs[s] = (t, off)
                off += slices[s][1]

        # head-critical loads, k-chunked, interleaved across both DGE
        # queues: scalar carries embT chunks, sync carries slice-0 chunks
        # with the next blocks threaded between so each slice's data lands
        # just ahead of the PE reaching it.
        ks0 = cst.tile([128, KT, 512], BF16, tag="ks0")
        ksbs[0] = (ks0, 0)
        for k in (0, 1):
            nc.scalar.dma_start(out=embT_sb[:, k, :], in_=embR[:, k, :])
            nc.sync.dma_start(out=ks0[:, k, :], in_=kernR[:, k, 0:512])
        load_block(1)
        for k in (2, 3):
            nc.scalar.dma_start(out=embT_sb[:, k, :], in_=embR[:, k, :])
            nc.sync.dma_start(out=ks0[:, k, :], in_=kernR[:, k, 0:512])
        load_block(2)
        load_block(3)

        # PE p-state warm-up while the first loads stream in
        wps = ps.tile([128, 512], F32, space="PSUM", tag="po")
        for _ in range(N_WARMUP):
            nc.tensor.matmul(
                wps[:], zeros[:, :128], zeros[:, 128:], start=True, stop=True
            )

        for bi in range(NBLK):
            for si in blocks[bi]:
                c0, W = slices[si]
                ksb, off = ksbs[si]
                o_sb = op_.tile([128, BT, 512], BF16, tag="o")
                psts = [
                    ps.tile(
                        [128, 512], F32, space="PSUM", tag="po", name=f"po{b}"
                    )
                    for b in range(BT)
                ]
                if si == 0:
                    # k-major: stream against the arriving k-chunks
                    for k in range(KT):
                        for b in range(BT):
                            nc.tensor.matmul(
                                psts[b][:, :W],
                                embT_sb[:, k, b * 128 : (b + 1) * 128],
                                ksb[:, k, off : off + W],
                                start=(k == 0),
                                stop=(k == KT - 1),
                            )
                    for b in range(BT):
                        nc.vector.tensor_scalar(
                            o_sb[:, b, :W], psts[b][:, :W], -CLIP, CLIP,
                            op0=AL.max, op1=AL.min,
                        )
                else:
                    # b-major: each B-tile's eviction overlaps the next tile
                    for b in range(BT):
                        for k in range(KT):
                            nc.tensor.matmul(
                                psts[b][:, :W],
                                embT_sb[:, k, b * 128 : (b + 1) * 128],
                                ksb[:, k, off : off + W],
                                start=(k == 0),
                                stop=(k == KT - 1),
                            )
                        nc.vector.tensor_scalar(
                            o_sb[:, b, :W], psts[b][:, :W], -CLIP, CLIP,
                            op0=AL.max, op1=AL.min,
                        )
                nc.sync.dma_start(
                    out=out3[:, :, c0 : c0 + W], in_=o_sb[:, :, :W]
                )
            if bi + 4 < NBLK:
                load_block(bi + 4)


def _build():
    nc = bacc.Bacc(
        "TRN2", target_bir_lowering=False, debug=False, num_devices=NCORES
    )
    embT_h = nc.dram_tensor("embT", [EMB, B], BF16, kind="ExternalInput")
    kern_h = nc.dram_tensor("kern", [EMB, CS], BF16, kind="ExternalInput")
    out_h = nc.dram_tensor("out", [BT * 128 * CS, 1], BF16, kind="ExternalOutput")
    with tile.TileContext(nc) as tc:
        _emit(nc, tc, embT_h, kern_h, out_h)
    nc.compile()
    return nc


_NC = None
_RUN_KW = {}
_LAST_RES = None


def _get_nc():
    global _NC
    if _NC is None:
        _NC = _build()
    return _NC


def _prep_inputs(embbedings, norms, label, kernel):
    import ml_dtypes

    bf16 = ml_dtypes.bfloat16
    emb_f = np.asarray(embbedings, dtype=np.float32)
    kern_f = np.asarray(kernel, dtype=np.float32)
    col_norm = np.sqrt(np.einsum("ec,ec->c", kern_f, kern_f))  # [C]
    knS = kern_f * (S / col_norm)[None, :]
    kern_pad = np.zeros((EMB, CS * NCORES), dtype=bf16)
    kern_pad[:, :C] = knS.astype(bf16)
    embT = np.ascontiguousarray(emb_f.T).astype(bf16)
    in_maps = []
    for c in range(NCORES):
        in_maps.append(
            {
                "embT": embT,
                "kern": np.ascontiguousarray(kern_pad[:, c * CS : (c + 1) * CS]),
            }
        )
    return in_maps, col_norm


def _host_fixup(out, embbedings, norms, label, kernel, col_norm):
    """Patch out[i, label_i] with the margin-adjusted value (reference math)."""
    emb_f = np.asarray(embbedings, dtype=np.float32)
    kern_f = np.asarray(kernel, dtype=np.float32)
    lab = np.asarray(label).astype(np.int64).reshape(B)
    v = np.clip(np.asarray(norms, dtype=np.float32).reshape(B), 0.001, 100.0)

    cnt = np.bincount(lab, minlength=C).astype(np.float32)
    ssum = np.bincount(lab, weights=v, minlength=C).astype(np.float32)
    ssq = np.bincount(lab, weights=v * v, minlength=C).astype(np.float32)
    n = cnt[lab]
    mean = ssum[lab] / n
    var = (ssq[lab] - n * mean * mean) / np.maximum(n - 1.0, 1.0)
    std = np.sqrt(np.maximum(var, 0.0))
    res = np.where(n > 2.0, (v - mean) / (std + EPS), (v - mean) / 20.0)
    ms = np.clip(res * H, -1.0, 1.0)

    cos = np.einsum("be,eb->b", emb_f, kern_f[:, lab]) / col_norm[lab]
    t = np.clip(cos, -1.0 + EPS, 1.0 - EPS)
    theta_m = np.clip(np.arccos(t) - MARG * ms, EPS, math.pi - EPS)
    val = (np.cos(theta_m) - (MARG + MARG * ms)) * S
    out[np.arange(B), lab] = val.astype(np.float32)


def _run(in_maps, **kwargs):
    nc = _get_nc()
    kw = dict(_RUN_KW)
    kw.update(kwargs)
    return run_bass_kernel_spmd(nc, in_maps, core_ids=list(range(NCORES)), **kw)


def kernel(embbedings, norms, label, kernel):
    global _LAST_RES
    in_maps, col_norm = _prep_inputs(embbedings, norms, label, kernel)
    res = _run(in_maps)
    _LAST_RES = res
    parts = [
        res.results[c]["out"].reshape(B, CS).astype(np.float32)
        for c in range(NCORES)
    ]
    out = np.concatenate(parts, axis=1)[:, :C]
    _host_fixup(out, embbedings, norms, label, kernel, col_norm)
    return out
